# revision 13
# baseline (speedup 1.0000x reference)
"""BoxFilter (9x9 box-sum, clamped borders) Trainium2 Bass kernel.

Input  x: [16, 3, 1024, 1024] f32, r=4 (hardcoded).
Output y: same shape; y[b,c,i,j] = sum of x[b,c,u,v] over the
(2r+1)x(2r+1) window centered at (i,j), clipped to the image bounds
(exactly what the reference's cumsum+diff computes).

Sharding: pure data parallel over 8 cores, 6 of the 48 images each.

Per-core pipeline (per image, 9 overlapping 128-row slabs):
  - The host splits x into bf16 hi/lo parts (x ~= hi + lo, residual
    <= 2^-18 relative) packed as [H, 2, W], so the H-direction matmul
    runs at bf16 speed (1 cycle/row vs 4 for fp32) with fp32-grade
    accuracy: band weights are exact 0/1 and PSUM accumulates in fp32.
  - H direction: banded 0/1 bf16 matmul on the TensorEngine. Slabs are
    chosen so each output-row block (124/120/60 rows) only needs input
    rows inside one 128-row slab -> 2 accumulating matmuls (hi+lo) per
    512-column PSUM bank, no cross-slab accumulation.
  - PSUM -> SBUF copies on the ScalarEngine, into a tile with 9
    leading zero columns.
  - W direction: running 9-window sum on the VectorEngine via ONE
    tensor_tensor_scan: state = (y[t] + state) - y[t-9] gives
    box_end[t] = sum_{k=max(0,t-8)}^{t} y[k] (the zero pad makes the
    left clamp automatic). Output col j (j<=W-r-1) is box_end[j+r].
  - The last r columns come from a tiny GpSimd scan seeded with
    box_end[W-1]: state' = (0 + state) - y[t] walks the right clamp.
"""

import os
import numpy as np
import ml_dtypes

from concourse import bass, mybir, tile, bacc
from concourse.bass_utils import run_bass_kernel_spmd

F32 = mybir.dt.float32
BF16 = mybir.dt.bfloat16
H, W = 1024, 1024
N_CORES = 8
IPC = 6  # images per core: (16*3)/8
R = 4
D = 2 * R + 1  # 9

# slabs: (row0, nrows, out0, nouts, band_col)
_SLABS = (
    [(0, 128, 0, 124, 0)]
    + [(120 * i, 128, 120 * i + 4, 120, 124) for i in range(1, 8)]
    + [(960, 64, 964, 60, 244)]
)
_BAND_COLS = 304  # 124 + 120 + 60


def _band_matrix() -> np.ndarray:
    bands = np.zeros((128, _BAND_COLS), ml_dtypes.bfloat16)
    for row0, nrows, out0, nouts, bc in (_SLABS[0], _SLABS[1], _SLABS[8]):
        for j in range(nouts):
            h_out = out0 + j
            lo = max(0, h_out - R) - row0
            hi = min(H - 1, h_out + R) - row0
            bands[lo : hi + 1, bc + j] = 1.0
    return bands


_CACHE: dict = {}

# Set by the most recent kernel() call (for test harnesses).
LAST_RESULTS = None


def _build():
    nc = bacc.Bacc(
        "TRN2", target_bir_lowering=False, debug=False, enable_asserts=False
    )
    # hi/lo packed per row: x_hl[img, h, 0, :] = bf16 hi, [.., 1, :] = lo
    xhl_d = nc.dram_tensor("x_hl", [IPC, H, 2, W], BF16, kind="ExternalInput").ap()
    bands_d = nc.dram_tensor(
        "bands", [128, _BAND_COLS], BF16, kind="ExternalInput"
    ).ap()
    y_d = nc.dram_tensor("y", [IPC, H, W], F32, kind="ExternalOutput").ap()

    ADD = mybir.AluOpType.add
    SUB = mybir.AluOpType.subtract

    with tile.TileContext(nc) as tc:
        with (
            tc.tile_pool(name="const", bufs=1) as const_pool,
            tc.tile_pool(name="xin", bufs=12) as in_pool,
            tc.tile_pool(name="ps", bufs=8, space="PSUM") as ps_pool,
            tc.tile_pool(name="yrow", bufs=8) as y_pool,
            tc.tile_pool(name="box", bufs=8) as box_pool,
        ):
            bands_t = const_pool.tile([128, _BAND_COLS], BF16)
            nc.sync.dma_start(bands_t[:], bands_d[:])

            slab_idx = 0
            for img in range(IPC):
                for row0, nrows, out0, nouts, bc in _SLABS:
                    # [nrows, 2, 1024] -> [nrows part, 2048 free]: hi cols
                    # [0:1024), lo cols [1024:2048)
                    xhl = in_pool.tile([128, 2 * W], BF16, tag="xhl")
                    nc.gpsimd.dma_start(
                        xhl[:nrows].rearrange("p (two w) -> p two w", two=2),
                        xhl_d[img, row0 : row0 + nrows, :, :],
                    )

                    # yt: [0:9) zeros, [9:1033) = H-filtered rows, [1033:1037)
                    # zeros (drives the right-border steps of the merged scan)
                    yt = y_pool.tile([128, W + D + R], F32, tag="yrow")
                    if slab_idx < 8:
                        # First 8 allocations occupy 8 distinct pool slots;
                        # pads are never overwritten, so zero them once per
                        # physical buffer (full 128 partitions).
                        nc.vector.memset(yt[:, 0:D], 0.0)
                        nc.vector.memset(yt[:, D + W : D + W + R], 0.0)

                    band_ap = bands_t[:nrows, bc : bc + nouts]
                    for h in range(2):
                        ps = ps_pool.tile([128, 512], F32, tag="ps")
                        nc.tensor.matmul(
                            ps[:nouts],
                            lhsT=band_ap,
                            rhs=xhl[:nrows, h * 512 : (h + 1) * 512],
                            start=True,
                            stop=False,
                        )
                        nc.tensor.matmul(
                            ps[:nouts],
                            lhsT=band_ap,
                            rhs=xhl[:nrows, W + h * 512 : W + (h + 1) * 512],
                            start=False,
                            stop=True,
                        )
                        nc.scalar.copy(
                            yt[:nouts, D + h * 512 : D + (h + 1) * 512],
                            ps[:nouts],
                        )

                    # Merged scan: state = (y[t] + state) - y[t-9] over 1028
                    # steps. Steps 1024..1027 read data0 = 0 (tail pad) and
                    # data1 = y[W-9..W-6], which walks the right clamp down
                    # from box_end[W-1]. Output col j (j < W-r) = bx[j+r].
                    bx = box_pool.tile([128, W + R], F32, tag="box")
                    nc.vector.tensor_tensor_scan(
                        bx[:nouts, 0 : W + R],
                        yt[:nouts, D : D + W + R],
                        yt[:nouts, 0 : W + R],
                        0.0,
                        op0=ADD,
                        op1=SUB,
                    )
                    nc.sync.dma_start(
                        y_d[img, out0 : out0 + nouts, :], bx[:nouts, R : R + W]
                    )
                    slab_idx += 1

    nc.compile()
    return nc


def kernel(x: np.ndarray, r) -> np.ndarray:
    global LAST_RESULTS
    x = np.asarray(x, dtype=np.float32)
    assert x.shape == (16, 3, H, W), x.shape
    assert int(r) == R, r

    nc = _CACHE.get("nc")
    if nc is None:
        nc = _CACHE["nc"] = _build()

    xr = x.reshape(N_CORES, IPC, H, W)
    x_hi = xr.astype(ml_dtypes.bfloat16)
    x_lo = (xr - x_hi.astype(np.float32)).astype(ml_dtypes.bfloat16)
    x_hl = np.stack([x_hi, x_lo], axis=3)  # [cores, IPC, H, 2, W]
    bands = _band_matrix()
    in_maps = [
        {"x_hl": np.ascontiguousarray(x_hl[c]), "bands": bands}
        for c in range(N_CORES)
    ]

    trace = bool(int(os.environ.get("BOX_TRACE", "0")))
    tmpdir = os.environ.get("BOX_TRACE_DIR") or None
    if tmpdir:
        os.makedirs(tmpdir, exist_ok=True)
    res = run_bass_kernel_spmd(
        nc, in_maps, list(range(N_CORES)), trace=trace, tmpdir=tmpdir
    )
    LAST_RESULTS = res
    y = np.stack([res.results[c]["y"] for c in range(N_CORES)])
    return y.reshape(16, 3, H, W)
